# revision 11
# baseline (speedup 1.0000x reference)
"""Trainium2 Bass kernel for nn_AttentionBlock (GroupNorm + 4-head attention + proj).

Sharding: 8 cores = (batch b in {0,1}) x (t-quarter tq in {0..3}).
Each core computes, for its batch and its 1024-wide query slice:
  - GroupNorm stats over the full [256, 4096] batch slab (on device)
  - q for its t-quarter, k/v for the full sequence (all 4 heads)
  - flash-style attention in S^T orientation (softmax over the partition dim,
    denominator obtained via a ones-column in the AV matmul)
  - proj + bias + residual for its t-quarter -> out [256, 1024]
Host only slices inputs per core and concatenates the 8 output tiles.

Precision: matmul operands bf16 (PE streams 1 row/cycle), every accumulation
fp32 in PSUM, softmax scores and exp input fp32, softmax denominator and
normalization fp32.
"""

import os
import sys

for _p in ("/opt/trn_rl_repo", "/opt/pypackages"):
    if _p not in sys.path and os.path.isdir(_p):
        sys.path.append(_p)

import numpy as np

import concourse.bass as bass
import concourse.tile as tile
from concourse import bacc, bass2jax, mybir

# ---------------- problem constants ----------------
B, C, HS, WS = 2, 256, 64, 64
T = HS * WS            # 4096
NH = 4                 # heads
CH = C // NH           # 64 channels / head
GROUPS = 32
GSIZE = C // GROUPS    # 8 channels / group
EPS = 1e-5
SCALE = CH ** -0.25

NCORES = 8
TQ = T // 4            # 1024 query positions per core
SB = 128               # s-block (key positions per QK matmul)
NSB = T // SB          # 32 s-blocks
TT = 512               # t-tile width for QK/AV
NTT = TQ // TT         # 2 t-tiles per core

F32 = mybir.dt.float32
BF16 = mybir.dt.bfloat16

# s-group pattern: sizes of consecutive s-block groups (bf16 score tiles, each
# group <= 6 s-blocks = 3 PSUM banks).  Sums to 32.
SGROUPS = [6, 6, 6, 6, 4, 4]
assert sum(SGROUPS) == NSB


def build_nc():
    nc = bacc.Bacc("TRN2", target_bir_lowering=False, debug=False)

    # ---- I/O ----
    x_ext = nc.declare_dram_parameter("x", [C, T], F32, isOutput=False)
    xq_ext = nc.declare_dram_parameter("xq", [C, TQ], F32, isOutput=False)
    wqkvT_ext = nc.declare_dram_parameter("wqkvT", [C, 3 * C], F32, isOutput=False)
    bqkv_ext = nc.declare_dram_parameter("bqkv", [3 * C], F32, isOutput=False)
    bv_row_ext = nc.declare_dram_parameter("bv_row", [1, C], F32, isOutput=False)
    wprojT_ext = nc.declare_dram_parameter("wprojT", [C, C], F32, isOutput=False)
    pb_row_ext = nc.declare_dram_parameter("pb_row", [1, C], F32, isOutput=False)
    normw_ext = nc.declare_dram_parameter("normw", [C], F32, isOutput=False)
    normb_ext = nc.declare_dram_parameter("normb", [C], F32, isOutput=False)
    gind_ext = nc.declare_dram_parameter("gind", [128, 16], F32, isOutput=False)
    gindT_ext = nc.declare_dram_parameter("gindT", [16, 128], F32, isOutput=False)
    out_ext = nc.declare_dram_parameter("out", [C, TQ], F32, isOutput=True)

    with tile.TileContext(nc) as tc:
        with (
            tc.tile_pool(name="sing", bufs=1) as sing,
            tc.tile_pool(name="ptp", bufs=3) as ptp,
            tc.tile_pool(name="tmp", bufs=3) as tmp,
            tc.tile_pool(name="psS", bufs=2, space="PSUM") as psS,
            tc.tile_pool(name="psV", bufs=2, space="PSUM") as psV,
        ):
            # ---------------- persistent SBUF ----------------
            sb_x = sing.tile([128, 2, T], F32)         # raw x (stats)
            sb_xq = sing.tile([128, 2, TQ], F32)       # raw x quarter (residual)
            sb_xn = sing.tile([128, 2, T], BF16)       # normalized x
            sb_xnq = sing.tile([128, 2, TQ], BF16)     # normalized x quarter
            k2 = sing.tile([128, 2, T], BF16)          # k in psum-block layout
            k2s = sing.tile([128, 2, T], BF16)         # same, halves swapped
            q2 = sing.tile([128, 2, TQ], BF16)
            q2s = sing.tile([128, 2, TQ], BF16)
            vT_sb = sing.tile([128, NSB, NH, CH + 1], BF16)  # v^T (+ones col)
            a_sb = sing.tile([128, 2, TQ], BF16)       # attention out (channels)
            out_sb = sing.tile([128, 2, TQ], F32)
            w_qkvT = sing.tile([128, 2, 3 * C], F32)   # fp32 staging
            w_projT = sing.tile([128, 2, C], F32)
            w_qkv_bf = sing.tile([128, 2, 3 * C], BF16)
            w_proj_bf = sing.tile([128, 2, C], BF16)
            bias_qk = sing.tile([128, 4, 1], F32)      # [qblk0,qblk1,kblk0,kblk1]
            bv_row = sing.tile([1, C], F32)
            pb_row = sing.tile([1, C], F32)
            bv_bf = sing.tile([1, C], BF16)
            pb_bf = sing.tile([1, C], BF16)
            normw = sing.tile([128, 2, 1], F32)
            normb = sing.tile([128, 2, 1], F32)
            gind = sing.tile([128, 16], F32)
            gindT = sing.tile([16, 128], F32)
            ones_bf = sing.tile([128, TT], BF16)
            ones_f32 = sing.tile([128, 64], F32)
            eps16 = sing.tile([16, 1], F32)
            ga = sing.tile([128, 2, 2], F32)           # per-channel [A, B] affine

            # ---------------- input / constant DMAs ----------------
            # x first, in 1024-wide chunks so bn_stats can chase the DMA
            for cb in range(2):
                for ch in range(4):
                    nc.sync.dma_start(
                        out=sb_x[:, cb, ch * 1024 : (ch + 1) * 1024],
                        in_=x_ext[cb * 128 : (cb + 1) * 128,
                                  ch * 1024 : (ch + 1) * 1024],
                    )
            for cb in range(2):
                nc.sync.dma_start(
                    out=sb_xq[:, cb, :], in_=xq_ext[cb * 128 : (cb + 1) * 128, :]
                )
            nc.vector.memset(ones_bf, 1.0)
            nc.vector.memset(ones_f32, 1.0)
            nc.vector.memset(eps16, EPS)
            nc.vector.memset(vT_sb[:, :, :, CH : CH + 1], 1.0)
            nc.sync.dma_start(out=gind, in_=gind_ext[:, :])
            nc.sync.dma_start(out=gindT, in_=gindT_ext[:, :])
            nc.sync.dma_start(out=bv_row, in_=bv_row_ext[:, :])
            nc.sync.dma_start(out=pb_row, in_=pb_row_ext[:, :])
            nc.vector.tensor_copy(bv_bf, bv_row)
            nc.vector.tensor_copy(pb_bf, pb_row)
            for cb in range(2):
                nc.sync.dma_start(
                    out=w_qkvT[:, cb, :], in_=wqkvT_ext[cb * 128 : (cb + 1) * 128, :]
                )
                nc.sync.dma_start(
                    out=w_projT[:, cb, :], in_=wprojT_ext[cb * 128 : (cb + 1) * 128, :]
                )
                nc.vector.tensor_copy(w_qkv_bf[:, cb, :], w_qkvT[:, cb, :])
                nc.vector.tensor_copy(w_proj_bf[:, cb, :], w_projT[:, cb, :])
                nc.sync.dma_start(
                    out=normw[:, cb, 0], in_=normw_ext[cb * 128 : (cb + 1) * 128]
                )
                nc.sync.dma_start(
                    out=normb[:, cb, 0], in_=normb_ext[cb * 128 : (cb + 1) * 128]
                )
            # q/k biases in block order: q blocks (h0,h1),(h2,h3) then k blocks
            for j in range(2):
                nc.sync.dma_start(
                    out=bias_qk[:, j, 0], in_=bqkv_ext[j * 128 : (j + 1) * 128]
                )
                nc.sync.dma_start(
                    out=bias_qk[:, 2 + j, 0],
                    in_=bqkv_ext[C + j * 128 : C + (j + 1) * 128],
                )

            # ---------------- GroupNorm stats ----------------
            for cb in range(2):
                stats = tmp.tile([128, 8, 6], F32, tag="bnst")
                for kk in range(8):
                    nc.vector.bn_stats(
                        out=stats[:, kk, :], in_=sb_x[:, cb, kk * 512 : (kk + 1) * 512]
                    )
                mv = tmp.tile([128, 2], F32, tag="mv")
                nc.vector.bn_aggr(out=mv, in_=stats)
                # mv[:,1] := E[x^2]_c = var_c + mean_c^2
                msq = tmp.tile([128, 1], F32, tag="msq")
                nc.vector.tensor_mul(msq, mv[:, 0:1], mv[:, 0:1])
                nc.vector.tensor_add(mv[:, 1:2], mv[:, 1:2], msq)
                # group-aggregate: [16, 2] = gind^T @ [mean_c, E[x^2]_c] (avg /8)
                gstat = psV.tile([16, 2], F32, tag="v")
                nc.tensor.matmul(gstat, lhsT=gind, rhs=mv, start=True, stop=True)
                gs_s = tmp.tile([16, 2], F32, tag="gss")
                nc.vector.tensor_copy(gs_s, gstat)
                # var_g = E[x^2]_g - mean_g^2 ; rstd_g = 1/sqrt(var_g + eps)
                g_ms = tmp.tile([16, 1], F32, tag="gms")
                nc.vector.tensor_mul(g_ms, gs_s[:, 0:1], gs_s[:, 0:1])
                g_sr = tmp.tile([16, 2], F32, tag="gsr")  # [rstd_g, mean_g]
                nc.vector.tensor_sub(g_sr[:, 0:1], gs_s[:, 1:2], g_ms)
                nc.scalar.activation(
                    out=g_sr[:, 0:1],
                    in_=g_sr[:, 0:1],
                    func=mybir.ActivationFunctionType.Sqrt,
                    bias=eps16,
                    scale=1.0,
                )
                nc.vector.reciprocal(g_sr[:, 0:1], g_sr[:, 0:1])
                nc.vector.tensor_copy(g_sr[:, 1:2], gs_s[:, 0:1])
                # broadcast group->channel via matmul with indicator
                cstat = psB.tile([128, 2], F32, tag="b")  # [rstd_c, mean_c]
                nc.tensor.matmul(cstat, lhsT=gindT, rhs=g_sr, start=True, stop=True)
                # A = rstd*w ; Bb = normb - mean*A
                nc.vector.tensor_mul(ga[:, cb, 0:1], cstat[:, 0:1], normw[:, cb, :])
                mA = tmp.tile([128, 1], F32, tag="mA")
                nc.vector.tensor_mul(mA, cstat[:, 1:2], ga[:, cb, 0:1])
                nc.vector.tensor_sub(ga[:, cb, 1:2], normb[:, cb, :], mA)
                # xn = x*A + Bb  (bf16 out) ; same for the quarter copy
                nc.vector.tensor_scalar(
                    out=sb_xn[:, cb, :],
                    in0=sb_x[:, cb, :],
                    scalar1=ga[:, cb, 0:1],
                    scalar2=ga[:, cb, 1:2],
                    op0=mybir.AluOpType.mult,
                    op1=mybir.AluOpType.add,
                )
                nc.vector.tensor_scalar(
                    out=sb_xnq[:, cb, :],
                    in0=sb_xq[:, cb, :],
                    scalar1=ga[:, cb, 0:1],
                    scalar2=ga[:, cb, 1:2],
                    op0=mybir.AluOpType.mult,
                    op1=mybir.AluOpType.add,
                )

            # rotating psum allocator for the production phase
            pools2 = [psS, psV]
            tags2 = ["s", "v"]
            pcnt = 0

            def prod_psum(shape):
                nonlocal pcnt
                p = pools2[pcnt % 2].tile(shape, F32, tag=tags2[pcnt % 2])
                pcnt += 1
                return p

            # alternate evict engine: DVE and ACT (idle during production)
            ecnt = 0

            def evict(dst, psum, bias=None):
                nonlocal ecnt
                ecnt += 1
                if ecnt % 2 == 0:
                    if bias is None:
                        nc.vector.tensor_copy(dst, psum)
                    else:
                        nc.vector.tensor_scalar(
                            out=dst, in0=psum, scalar1=bias, scalar2=None,
                            op0=mybir.AluOpType.add,
                        )
                else:
                    if bias is None:
                        nc.scalar.activation(
                            out=dst, in_=psum,
                            func=mybir.ActivationFunctionType.Copy, scale=1.0,
                        )
                    else:
                        nc.scalar.activation(
                            out=dst, in_=psum,
                            func=mybir.ActivationFunctionType.Copy,
                            bias=bias, scale=1.0,
                        )

            # ---------------- q production (t-quarter) ----------------
            # weight columns pre-permuted: [q h0..h3 | k h0..h3 | v h0..h3]
            for mb in range(2):  # head pairs (h0,h1), (h2,h3)
                for nt in range(NTT):
                    pq = prod_psum([128, TT])
                    for cb in range(2):
                        nc.tensor.matmul(
                            pq,
                            lhsT=w_qkv_bf[:, cb, mb * 128 : (mb + 1) * 128],
                            rhs=sb_xnq[:, cb, nt * TT : (nt + 1) * TT],
                            start=(cb == 0),
                            stop=(cb == 1),
                        )
                    # + bias, evict into both duplicate halves
                    for hh in range(2):  # low/high psum half
                        h = mb * 2 + hh
                        dst = q_sb[hh * 64 : (hh + 1) * 64, h, nt * TT : (nt + 1) * TT]
                        nc.vector.tensor_scalar(
                            out=dst,
                            in0=pq[hh * 64 : (hh + 1) * 64, :],
                            scalar1=bias_qk[hh * 64 : (hh + 1) * 64, mb, :],
                            scalar2=None,
                            op0=mybir.AluOpType.add,
                        )
                    # duplicate across partition halves (DMA crosses partitions)
                    h0, h1 = mb * 2, mb * 2 + 1
                    nc.sync.dma_start(
                        out=q_sb[64:128, h0, nt * TT : (nt + 1) * TT],
                        in_=q_sb[0:64, h0, nt * TT : (nt + 1) * TT],
                    )
                    nc.sync.dma_start(
                        out=q_sb[0:64, h1, nt * TT : (nt + 1) * TT],
                        in_=q_sb[64:128, h1, nt * TT : (nt + 1) * TT],
                    )

            # ---------------- k production (full T) ----------------
            for mb in range(2):
                for nt in range(T // TT):
                    pk = prod_psum([128, TT])
                    for cb in range(2):
                        nc.tensor.matmul(
                            pk,
                            lhsT=w_qkv_bf[:, cb, C + mb * 128 : C + (mb + 1) * 128],
                            rhs=sb_xn[:, cb, nt * TT : (nt + 1) * TT],
                            start=(cb == 0),
                            stop=(cb == 1),
                        )
                    for hh in range(2):
                        h = mb * 2 + hh
                        dst = k_sb[hh * 64 : (hh + 1) * 64, h, nt * TT : (nt + 1) * TT]
                        nc.vector.tensor_scalar(
                            out=dst,
                            in0=pk[hh * 64 : (hh + 1) * 64, :],
                            scalar1=bias_qk[hh * 64 : (hh + 1) * 64, 2 + mb, :],
                            scalar2=None,
                            op0=mybir.AluOpType.add,
                        )
                    h0, h1 = mb * 2, mb * 2 + 1
                    nc.sync.dma_start(
                        out=k_sb[64:128, h0, nt * TT : (nt + 1) * TT],
                        in_=k_sb[0:64, h0, nt * TT : (nt + 1) * TT],
                    )
                    nc.sync.dma_start(
                        out=k_sb[0:64, h1, nt * TT : (nt + 1) * TT],
                        in_=k_sb[64:128, h1, nt * TT : (nt + 1) * TT],
                    )

            # ---------------- v^T production ----------------
            for tcn in range(T // 128):
                pv = prod_psum([128, C])
                for cb in range(2):
                    nc.tensor.matmul(
                        pv,
                        lhsT=sb_xn[:, cb, tcn * 128 : (tcn + 1) * 128],
                        rhs=w_qkv_bf[:, cb, 2 * C : 3 * C],
                        start=(cb == 0),
                        stop=False,
                    )
                # rank-1 bias add: ones^T @ bv_row
                nc.tensor.matmul(
                    pv,
                    lhsT=ones_bf[0:1, 0:128],
                    rhs=bv_bf,
                    start=False,
                    stop=True,
                )
                nc.vector.tensor_copy(
                    vT_sb[:, tcn, :, 0:CH],
                    pv.rearrange("p (h c) -> p h c", h=NH),
                )

            # ---------------- attention ----------------
            for tt in range(NTT):
                tsl = slice(tt * TT, (tt + 1) * TT)
                for h in range(NH):
                    av = psV.tile([CH + 1, TT], F32, tag="v")
                    s0 = 0
                    for gi, gsz in enumerate(SGROUPS):
                        pool, tg = (psA, "a") if gi % 2 == 0 else (psB, "b")
                        st = pool.tile([128, gsz, TT], F32, tag=tg)
                        # QK^T: pairs of s-blocks packed into both PE halves
                        for j in range(gsz):
                            s = s0 + j
                            half = j % 2
                            pr = slice(half * 64, (half + 1) * 64)
                            nc.tensor.matmul(
                                st[:, j, :],
                                lhsT=k_sb[pr, h, s * SB : (s + 1) * SB],
                                rhs=q_sb[pr, h, tsl],
                                start=True,
                                stop=True,
                            )
                        # exp over the whole group straight out of PSUM
                        pt = ptp.tile([128, 4, TT], BF16, tag="pt")
                        nc.scalar.activation(
                            out=pt[:, 0:gsz, :].rearrange("p g t -> p (g t)"),
                            in_=st.rearrange("p g t -> p (g t)"),
                            func=mybir.ActivationFunctionType.Exp,
                            scale=1.0,
                        )
                        # AV accumulation (ones column yields the softmax denom)
                        for j in range(gsz):
                            s = s0 + j
                            nc.tensor.matmul(
                                av,
                                lhsT=vT_sb[:, s, h, :],
                                rhs=pt[:, j, :],
                                start=(s == 0),
                                stop=(s == NSB - 1),
                            )
                        s0 += gsz
                    # normalize: r = 1/l ; broadcast r via K=1 matmul ; a = av*r
                    r65 = tmp.tile([65, TT], F32, tag="r65")
                    nc.vector.reciprocal(r65[64:65, :], av[CH : CH + 1, :])
                    rb = psB.tile([64, TT], F32, tag="b")
                    nc.tensor.matmul(
                        rb,
                        lhsT=ones_f32[64:65, 0:64],
                        rhs=r65[64:65, :],
                        start=True,
                        stop=True,
                    )
                    rb_s = tmp.tile([64, TT], F32, tag="rbs")
                    nc.vector.tensor_copy(rb_s, rb)
                    a_t = tmp.tile([64, TT], BF16, tag="atmp")
                    nc.vector.tensor_mul(a_t, av[0:CH, :], rb_s)
                    # place head channels: h0 -> [0:64, blk0], h1 -> [64:128, blk0]...
                    nc.sync.dma_start(
                        out=a_sb[(h % 2) * 64 : (h % 2) * 64 + 64, h // 2, tsl],
                        in_=a_t,
                    )

                # ---------------- proj + bias + residual for this t-tile ----
                for mb in range(2):
                    pp = (psA if mb == 0 else psB).tile(
                        [128, TT], F32, tag=("a" if mb == 0 else "b")
                    )
                    for cb in range(2):
                        nc.tensor.matmul(
                            pp,
                            lhsT=w_proj_bf[:, cb, mb * 128 : (mb + 1) * 128],
                            rhs=a_sb[:, cb, tsl],
                            start=(cb == 0),
                            stop=False,
                        )
                    nc.tensor.matmul(
                        pp,
                        lhsT=pb_bf[0:1, mb * 128 : (mb + 1) * 128],
                        rhs=ones_bf[0:1, 0:TT],
                        start=False,
                        stop=True,
                    )
                    nc.vector.tensor_add(
                        out_sb[:, mb, tsl], pp, sb_xq[:, mb, tsl]
                    )

            # ---------------- store ----------------
            for cb in range(2):
                nc.sync.dma_start(
                    out=out_ext[cb * 128 : (cb + 1) * 128, :], in_=out_sb[:, cb, :]
                )

    nc.compile()
    return nc


# ---------------- host side ----------------

def _prep_consts(qkv_w, qkv_b, proj_w, proj_b, norm_w, norm_b):
    qkv_w = np.asarray(qkv_w, np.float32)
    qkv_b = np.asarray(qkv_b, np.float32)
    # permute rows from per-head [q|k|v] interleave to [all q | all k | all v],
    # heads in order; fold the ch**-0.25 score scale into q and k
    perm = np.concatenate(
        [np.arange(NH)[:, None] * (3 * CH) + off + np.arange(CH)[None, :]
         for off in (0, CH, 2 * CH)]
    ).reshape(3 * C)
    wp = qkv_w[perm].copy()
    bp = qkv_b[perm].copy()
    wp[: 2 * C] *= SCALE
    bp[: 2 * C] *= SCALE
    gind = np.zeros((128, 16), np.float32)
    gindT = np.zeros((16, 128), np.float32)
    for p in range(128):
        gind[p, p // GSIZE] = 1.0 / GSIZE
        gindT[p // GSIZE, p] = 1.0
    return {
        "wqkvT": np.ascontiguousarray(wp.T),
        "bqkv": bp,
        "bv_row": np.ascontiguousarray(bp[2 * C :][None, :]),
        "wprojT": np.ascontiguousarray(np.asarray(proj_w, np.float32).T),
        "pb_row": np.ascontiguousarray(np.asarray(proj_b, np.float32)[None, :]),
        "normw": np.asarray(norm_w, np.float32),
        "normb": np.asarray(norm_b, np.float32),
        "gind": gind,
        "gindT": gindT,
    }


def _make_in_maps(x, norm_w, norm_b, qkv_w, qkv_b, proj_w, proj_b):
    x = np.asarray(x, np.float32)
    consts = _prep_consts(qkv_w, qkv_b, proj_w, proj_b, norm_w, norm_b)
    xf = x.reshape(B, C, T)
    in_maps = []
    for core in range(NCORES):
        b, tq = core // 4, core % 4
        m = dict(consts)
        m["x"] = np.ascontiguousarray(xf[b])
        m["xq"] = np.ascontiguousarray(xf[b][:, tq * TQ : (tq + 1) * TQ])
        in_maps.append(m)
    return in_maps


def _assemble(results):
    out = np.empty((B, C, T), np.float32)
    for core in range(NCORES):
        b, tq = core // 4, core % 4
        out[b][:, tq * TQ : (tq + 1) * TQ] = results[core]["out"]
    return out.reshape(B, C, HS, WS)


def kernel(x, norm_w, norm_b, qkv_w, qkv_b, proj_w, proj_b):
    in_maps = _make_in_maps(x, norm_w, norm_b, qkv_w, qkv_b, proj_w, proj_b)
    nc = build_nc()
    results = bass2jax.run_bass_via_pjrt(nc, in_maps, n_cores=NCORES)
    return _assemble(results)


if __name__ == "__main__":
    rng = np.random.default_rng(0)
    out = kernel(
        rng.standard_normal((B, C, HS, WS), np.float32),
        np.ones(C, np.float32),
        np.zeros(C, np.float32),
        rng.standard_normal((3 * C, C), np.float32) * C**-0.5,
        rng.standard_normal(3 * C, np.float32) * 0.02,
        rng.standard_normal((C, C), np.float32) * C**-0.5,
        rng.standard_normal(C, np.float32) * 0.02,
    )
    print(out.shape, float(np.abs(out).max()))
